# revision 17
# baseline (speedup 1.0000x reference)
"""MinGRU Trainium2 kernel.

Reference computation (per batch b):
    z   = sigmoid(x @ Wz^T + bz)          [T, D]
    ht  = x @ Wh^T + bh                   [T, D]
    h_t = (1 - z_t) * h_{t-1} + z_t * ht_t    (scan over t, h_{-1} = h_prev)
    returns (outputs [B,T,D], h_last [B,D])

Strategy:
  - Data parallel: batch b -> core b (B = 8 = n_cores).
  - Everything on-device lives in TRANSPOSED layout [D, T] (features on
    partitions, time on the free axis):
      * matmuls compute Zpre^T = Wz @ x^T directly (lhsT = Wz^T block,
        rhs = x^T block), accumulating K=1024 in 8 steps of 128 in PSUM.
      * the recurrence is ONE tensor_tensor_scan instruction per
        [128, TC] tile: state = a*state + b along the free (time) axis.
  - The host pre-transposes x and the weights (numpy, free relative to
    device time) and transposes the [D, T] result back to [T, D].
  - Matmul inputs are cast to fp16 on the host (same PE speed as bf16,
    measured, but 10 mantissa bits; PSUM accumulation stays fp32);
    everything after the matmul is fp32.
"""

import time

import numpy as np
import ml_dtypes

import concourse.bass as bass  # noqa: F401  (bass types used via tile/bacc)
import concourse.mybir as mybir
import concourse.tile as tile
from contextlib import ExitStack
from concourse import bacc
from concourse.bass_utils import run_bass_kernel_spmd

# Problem sizes (hardcoded per contract).
B, T, D = 8, 4096, 1024
P = 128            # SBUF partitions
ND = D // P        # 8 output-feature blocks
NK = D // P        # 8 contraction blocks
TC = 512           # time-chunk = matmul free dim = one fp32 PSUM bank
NT = T // TC       # 8 time chunks

F32 = mybir.dt.float32
BF16 = mybir.dt.bfloat16

# Matmul input precision: "bf16", "fp16", or "f32r" (fp32 data, fast PE mode).
MM_DTYPE = "fp16"


def build(nt: int = NT, repeat: int = 1):
    """Build the Bass module. All 8 cores run the same program (SPMD).

    repeat > 1 re-runs the whole compute loop over the same data
    (timing-measurement variant; output is overwritten each pass)."""
    t_total = nt * TC
    nc = bacc.Bacc("TRN2", target_bir_lowering=False, debug=False, num_devices=8)

    MMDT = {
        "bf16": BF16,
        "fp16": mybir.dt.float16,
        "f32r": mybir.dt.float32r,
    }[MM_DTYPE]

    xT = nc.dram_tensor("xT", [D, t_total], MMDT, kind="ExternalInput")
    wzT = nc.dram_tensor("wzT", [D, D], MMDT, kind="ExternalInput")   # [k, d]
    whT = nc.dram_tensor("whT", [D, D], MMDT, kind="ExternalInput")   # [k, d]
    bz = nc.dram_tensor("bz", [P, ND], F32, kind="ExternalInput")     # bz[p,j] = Wz_b[j*128+p]
    nbz = nc.dram_tensor("nbz", [P, ND], F32, kind="ExternalInput")   # -Wz_b
    bh = nc.dram_tensor("bh", [P, ND], F32, kind="ExternalInput")
    h0 = nc.dram_tensor("h0", [P, ND], F32, kind="ExternalInput")
    outT = nc.dram_tensor("outT", [D, t_total], F32, kind="ExternalOutput")

    with tile.TileContext(nc) as tc, ExitStack() as ctx:
        consts = ctx.enter_context(tc.tile_pool(name="consts", bufs=1))
        wpool = ctx.enter_context(tc.tile_pool(name="w", bufs=1))
        xpool = ctx.enter_context(tc.tile_pool(name="x", bufs=3))
        zpool = ctx.enter_context(tc.tile_pool(name="z", bufs=4))
        apool = ctx.enter_context(tc.tile_pool(name="a", bufs=4))
        bpool = ctx.enter_context(tc.tile_pool(name="b", bufs=4))
        hpool = ctx.enter_context(tc.tile_pool(name="h", bufs=2 * ND))
        pspool = ctx.enter_context(tc.tile_pool(name="ps", bufs=8, space="PSUM"))

        # --- prefetch chunk 0 of x ahead of the weights (cuts startup) ---
        xc0 = xpool.tile([P, NK, TC], MMDT, tag="xc")
        for kb in range(NK):
            nc.sync.dma_start(xc0[:, kb, :], xT[kb * P:(kb + 1) * P, 0:TC])

        # --- weights, transposed [k, d], one [128, D] tile per k-block ---
        wz_sb = []
        wh_sb = []
        for kb in range(NK):
            wz_t = wpool.tile([P, D], MMDT, tag=f"wz{kb}")
            nc.sync.dma_start(wz_t[:], wzT[kb * P:(kb + 1) * P, :])
            wz_sb.append(wz_t)
            wh_t = wpool.tile([P, D], MMDT, tag=f"wh{kb}")
            nc.sync.dma_start(wh_t[:], whT[kb * P:(kb + 1) * P, :])
            wh_sb.append(wh_t)

        # --- constants (small; after the bulk loads) ---
        bz_sb = consts.tile([P, ND], F32, tag="bz")
        nc.sync.dma_start(bz_sb[:], bz[:, :])
        nbz_sb = consts.tile([P, ND], F32, tag="nbz")
        nc.sync.dma_start(nbz_sb[:], nbz[:, :])
        bh_sb = consts.tile([P, ND], F32, tag="bh")
        nc.sync.dma_start(bh_sb[:], bh[:, :])
        h0_sb = consts.tile([P, ND], F32, tag="h0")
        nc.sync.dma_start(h0_sb[:], h0[:, :])

        # previous h tile per d-block (for scan chaining)
        h_prev_tiles = [None] * ND

        first = True
        for t in range(nt * repeat):
            t %= nt
            t0 = t * TC
            # x^T chunk: [128k, NK, TC]
            if first:
                xc = xc0
                first = False
            else:
                xc = xpool.tile([P, NK, TC], MMDT, tag="xc")
                for kb in range(NK):
                    nc.sync.dma_start(
                        xc[:, kb, :], xT[kb * P:(kb + 1) * P, t0:t0 + TC]
                    )

            for db in range(ND):
                dsl = slice(db * P, (db + 1) * P)
                # z-gate pre-activation: accumulate over k
                zp = pspool.tile([P, TC], F32, tag="ps")
                for kb in range(NK):
                    nc.tensor.matmul(
                        zp[:],
                        wz_sb[kb][:, dsl],
                        xc[:, kb, :],
                        start=(kb == 0),
                        stop=(kb == NK - 1),
                    )
                # h-tilde pre-activation
                hp = pspool.tile([P, TC], F32, tag="ps")
                for kb in range(NK):
                    nc.tensor.matmul(
                        hp[:],
                        wh_sb[kb][:, dsl],
                        xc[:, kb, :],
                        start=(kb == 0),
                        stop=(kb == NK - 1),
                    )

                # z = sigmoid(zp + bz);  a = 1 - z = sigmoid(-zp - bz)
                z_t = zpool.tile([P, TC], F32, tag="z")
                nc.scalar.activation(
                    z_t[:], zp[:], mybir.ActivationFunctionType.Sigmoid,
                    bias=bz_sb[:, db:db + 1], scale=1.0,
                )
                a_t = apool.tile([P, TC], F32, tag="a")
                nc.scalar.activation(
                    a_t[:], zp[:], mybir.ActivationFunctionType.Sigmoid,
                    bias=nbz_sb[:, db:db + 1], scale=-1.0,
                )
                # b = (hp + bh) * z
                b_t = bpool.tile([P, TC], F32, tag="b")
                nc.vector.scalar_tensor_tensor(
                    b_t[:], hp[:], bh_sb[:, db:db + 1], z_t[:],
                    op0=mybir.AluOpType.add, op1=mybir.AluOpType.mult,
                )
                # h_t = a*h_{t-1} + b  (scan along free/time axis)
                h_t = hpool.tile([P, TC], F32, tag="h")
                init = (
                    h0_sb[:, db:db + 1]
                    if h_prev_tiles[db] is None
                    else h_prev_tiles[db][:, TC - 1:TC]
                )
                nc.vector.tensor_tensor_scan(
                    h_t[:], a_t[:], b_t[:], init,
                    op0=mybir.AluOpType.mult, op1=mybir.AluOpType.add,
                )
                h_prev_tiles[db] = h_t
                nc.sync.dma_start(outT[dsl, t0:t0 + TC], h_t[:])

    nc.compile()
    return nc


def make_in_maps(x, h_prev, Wz_w, Wz_b, Wh_w, Wh_b):
    bf16 = {
        "bf16": ml_dtypes.bfloat16,
        "fp16": np.float16,
        "f32r": np.float32,
    }[MM_DTYPE]
    f32 = np.float32
    wzT = np.asarray(Wz_w).T.astype(bf16)          # [k, d], contiguous copy
    whT = np.asarray(Wh_w).T.astype(bf16)
    bz2 = np.asarray(Wz_b, f32).reshape(ND, P).T.copy()
    nbz2 = (-np.asarray(Wz_b, f32)).reshape(ND, P).T.copy()
    bh2 = np.asarray(Wh_b, f32).reshape(ND, P).T.copy()
    in_maps = []
    for b in range(B):
        in_maps.append({
            "xT": np.asarray(x[b]).T.astype(bf16),  # [D, T]
            "wzT": wzT,
            "whT": whT,
            "bz": bz2,
            "nbz": nbz2,
            "bh": bh2,
            "h0": np.asarray(h_prev[b], f32).reshape(ND, P).T.copy(),
        })
    return in_maps


def kernel(x, h_prev, Wz_w, Wz_b, Wh_w, Wh_b):
    x = np.asarray(x, np.float32)
    h_prev = np.asarray(h_prev, np.float32)
    Wz_w = np.asarray(Wz_w, np.float32)
    Wz_b = np.asarray(Wz_b, np.float32)
    Wh_w = np.asarray(Wh_w, np.float32)
    Wh_b = np.asarray(Wh_b, np.float32)
    in_maps = make_in_maps(x, h_prev, Wz_w, Wz_b, Wh_w, Wh_b)
    nc = build()
    res = None
    for attempt in range(2):
        try:
            res = run_bass_kernel_spmd(
                nc, in_maps, core_ids=list(range(B)), trace=False
            )
            break
        except Exception:
            if attempt == 1:
                raise
            # transient device errors have been observed to self-recover
            time.sleep(75)
    outputs = np.stack(
        [res.results[b]["outT"].T for b in range(B)]
    ).astype(np.float32)
    h_last = np.ascontiguousarray(outputs[:, -1, :])
    return outputs, h_last


# revision 19
# speedup vs baseline: 1.0098x; 1.0098x over previous
"""MinGRU Trainium2 kernel.

Reference computation (per batch b):
    z   = sigmoid(x @ Wz^T + bz)          [T, D]
    ht  = x @ Wh^T + bh                   [T, D]
    h_t = (1 - z_t) * h_{t-1} + z_t * ht_t    (scan over t, h_{-1} = h_prev)
    returns (outputs [B,T,D], h_last [B,D])

Strategy:
  - Data parallel: batch b -> core b (B = 8 = n_cores).
  - Everything on-device lives in TRANSPOSED layout [D, T] (features on
    partitions, time on the free axis):
      * matmuls compute Zpre^T = Wz @ x^T directly (lhsT = Wz^T block,
        rhs = x^T block), accumulating K=1024 in 8 steps of 128 in PSUM.
      * the recurrence is ONE tensor_tensor_scan instruction per
        [128, TC] tile: state = a*state + b along the free (time) axis.
  - The host pre-transposes x and the weights (numpy, free relative to
    device time) and transposes the [D, T] result back to [T, D].
  - Matmul inputs are cast to fp16 on the host (same PE speed as bf16,
    measured, but 10 mantissa bits; PSUM accumulation stays fp32);
    everything after the matmul is fp32.
"""

import time

import numpy as np
import ml_dtypes

import concourse.bass as bass  # noqa: F401  (bass types used via tile/bacc)
import concourse.mybir as mybir
import concourse.tile as tile
from contextlib import ExitStack
from concourse import bacc
from concourse.bass_utils import run_bass_kernel_spmd

# Problem sizes (hardcoded per contract).
B, T, D = 8, 4096, 1024
P = 128            # SBUF partitions
ND = D // P        # 8 output-feature blocks
NK = D // P        # 8 contraction blocks
TC = 512           # time-chunk = matmul free dim = one fp32 PSUM bank
NT = T // TC       # 8 time chunks

F32 = mybir.dt.float32
BF16 = mybir.dt.bfloat16

# Matmul input precision: "bf16", "fp16", or "f32r" (fp32 data, fast PE mode).
MM_DTYPE = "fp16"


def build(nt: int = NT, repeat: int = 1):
    """Build the Bass module. All 8 cores run the same program (SPMD).

    repeat > 1 re-runs the whole compute loop over the same data
    (timing-measurement variant; output is overwritten each pass)."""
    t_total = nt * TC
    nc = bacc.Bacc("TRN2", target_bir_lowering=False, debug=False, num_devices=8)

    MMDT = {
        "bf16": BF16,
        "fp16": mybir.dt.float16,
        "f32r": mybir.dt.float32r,
    }[MM_DTYPE]

    xT = nc.dram_tensor("xT", [D, t_total], MMDT, kind="ExternalInput")
    wzT = nc.dram_tensor("wzT", [D, D], MMDT, kind="ExternalInput")   # [k, d]
    whT = nc.dram_tensor("whT", [D, D], MMDT, kind="ExternalInput")   # [k, d]
    bz = nc.dram_tensor("bz", [P, ND], F32, kind="ExternalInput")     # bz[p,j] = Wz_b[j*128+p]
    nbz = nc.dram_tensor("nbz", [P, ND], F32, kind="ExternalInput")   # -Wz_b
    bh = nc.dram_tensor("bh", [P, ND], F32, kind="ExternalInput")
    h0 = nc.dram_tensor("h0", [P, ND], F32, kind="ExternalInput")
    outT = nc.dram_tensor("outT", [D, t_total], F32, kind="ExternalOutput")

    with tile.TileContext(nc) as tc, ExitStack() as ctx:
        consts = ctx.enter_context(tc.tile_pool(name="consts", bufs=1))
        wpool = ctx.enter_context(tc.tile_pool(name="w", bufs=1))
        xpool = ctx.enter_context(tc.tile_pool(name="x", bufs=3))
        zpool = ctx.enter_context(tc.tile_pool(name="z", bufs=4))
        apool = ctx.enter_context(tc.tile_pool(name="a", bufs=4))
        bpool = ctx.enter_context(tc.tile_pool(name="b", bufs=4))
        hpool = ctx.enter_context(tc.tile_pool(name="h", bufs=2 * ND))
        pspool = ctx.enter_context(tc.tile_pool(name="ps", bufs=8, space="PSUM"))

        # --- prefetch chunk 0 of x ahead of the weights (cuts startup) ---
        # one strided DMA per chunk: [p, kb, t] view of xT
        xTv = xT.rearrange("(kb p) t -> p kb t", p=P)
        xc0 = xpool.tile([P, NK, TC], MMDT, tag="xc")
        nc.sync.dma_start(xc0[:, :, :], xTv[:, :, 0:TC])

        # --- weights, transposed [k, d], one [128, D] tile per k-block ---
        wz_sb = []
        wh_sb = []
        for kb in range(NK):
            wz_t = wpool.tile([P, D], MMDT, tag=f"wz{kb}")
            nc.sync.dma_start(wz_t[:], wzT[kb * P:(kb + 1) * P, :])
            wz_sb.append(wz_t)
            wh_t = wpool.tile([P, D], MMDT, tag=f"wh{kb}")
            nc.sync.dma_start(wh_t[:], whT[kb * P:(kb + 1) * P, :])
            wh_sb.append(wh_t)

        # --- constants (small; after the bulk loads) ---
        bz_sb = consts.tile([P, ND], F32, tag="bz")
        nc.sync.dma_start(bz_sb[:], bz[:, :])
        nbz_sb = consts.tile([P, ND], F32, tag="nbz")
        nc.sync.dma_start(nbz_sb[:], nbz[:, :])
        bh_sb = consts.tile([P, ND], F32, tag="bh")
        nc.sync.dma_start(bh_sb[:], bh[:, :])
        h0_sb = consts.tile([P, ND], F32, tag="h0")
        nc.sync.dma_start(h0_sb[:], h0[:, :])

        # previous h tile per d-block (for scan chaining)
        h_prev_tiles = [None] * ND

        first = True
        for t in range(nt * repeat):
            t %= nt
            t0 = t * TC
            # x^T chunk: [128k, NK, TC]
            if first:
                xc = xc0
                first = False
            else:
                xc = xpool.tile([P, NK, TC], MMDT, tag="xc")
                nc.sync.dma_start(xc[:, :, :], xTv[:, :, t0:t0 + TC])

            for db in range(ND):
                dsl = slice(db * P, (db + 1) * P)
                # z-gate pre-activation: accumulate over k
                zp = pspool.tile([P, TC], F32, tag="ps")
                for kb in range(NK):
                    nc.tensor.matmul(
                        zp[:],
                        wz_sb[kb][:, dsl],
                        xc[:, kb, :],
                        start=(kb == 0),
                        stop=(kb == NK - 1),
                    )
                # h-tilde pre-activation
                hp = pspool.tile([P, TC], F32, tag="ps")
                for kb in range(NK):
                    nc.tensor.matmul(
                        hp[:],
                        wh_sb[kb][:, dsl],
                        xc[:, kb, :],
                        start=(kb == 0),
                        stop=(kb == NK - 1),
                    )

                # z = sigmoid(zp + bz);  a = 1 - z = sigmoid(-zp - bz)
                z_t = zpool.tile([P, TC], F32, tag="z")
                nc.scalar.activation(
                    z_t[:], zp[:], mybir.ActivationFunctionType.Sigmoid,
                    bias=bz_sb[:, db:db + 1], scale=1.0,
                )
                a_t = apool.tile([P, TC], F32, tag="a")
                nc.scalar.activation(
                    a_t[:], zp[:], mybir.ActivationFunctionType.Sigmoid,
                    bias=nbz_sb[:, db:db + 1], scale=-1.0,
                )
                # b = (hp + bh) * z
                b_t = bpool.tile([P, TC], F32, tag="b")
                nc.vector.scalar_tensor_tensor(
                    b_t[:], hp[:], bh_sb[:, db:db + 1], z_t[:],
                    op0=mybir.AluOpType.add, op1=mybir.AluOpType.mult,
                )
                # h_t = a*h_{t-1} + b  (scan along free/time axis)
                h_t = hpool.tile([P, TC], F32, tag="h")
                init = (
                    h0_sb[:, db:db + 1]
                    if h_prev_tiles[db] is None
                    else h_prev_tiles[db][:, TC - 1:TC]
                )
                nc.vector.tensor_tensor_scan(
                    h_t[:], a_t[:], b_t[:], init,
                    op0=mybir.AluOpType.mult, op1=mybir.AluOpType.add,
                )
                h_prev_tiles[db] = h_t
                nc.sync.dma_start(outT[dsl, t0:t0 + TC], h_t[:])

    nc.compile()
    return nc


def make_in_maps(x, h_prev, Wz_w, Wz_b, Wh_w, Wh_b):
    bf16 = {
        "bf16": ml_dtypes.bfloat16,
        "fp16": np.float16,
        "f32r": np.float32,
    }[MM_DTYPE]
    f32 = np.float32
    wzT = np.asarray(Wz_w).T.astype(bf16)          # [k, d], contiguous copy
    whT = np.asarray(Wh_w).T.astype(bf16)
    bz2 = np.asarray(Wz_b, f32).reshape(ND, P).T.copy()
    nbz2 = (-np.asarray(Wz_b, f32)).reshape(ND, P).T.copy()
    bh2 = np.asarray(Wh_b, f32).reshape(ND, P).T.copy()
    in_maps = []
    for b in range(B):
        in_maps.append({
            "xT": np.asarray(x[b]).T.astype(bf16),  # [D, T]
            "wzT": wzT,
            "whT": whT,
            "bz": bz2,
            "nbz": nbz2,
            "bh": bh2,
            "h0": np.asarray(h_prev[b], f32).reshape(ND, P).T.copy(),
        })
    return in_maps


def kernel(x, h_prev, Wz_w, Wz_b, Wh_w, Wh_b):
    x = np.asarray(x, np.float32)
    h_prev = np.asarray(h_prev, np.float32)
    Wz_w = np.asarray(Wz_w, np.float32)
    Wz_b = np.asarray(Wz_b, np.float32)
    Wh_w = np.asarray(Wh_w, np.float32)
    Wh_b = np.asarray(Wh_b, np.float32)
    in_maps = make_in_maps(x, h_prev, Wz_w, Wz_b, Wh_w, Wh_b)
    nc = build()
    res = None
    for attempt in range(2):
        try:
            res = run_bass_kernel_spmd(
                nc, in_maps, core_ids=list(range(B)), trace=False
            )
            break
        except Exception:
            if attempt == 1:
                raise
            # transient device errors have been observed to self-recover
            time.sleep(75)
    outputs = np.stack(
        [res.results[b]["outT"].T for b in range(B)]
    ).astype(np.float32)
    h_last = np.ascontiguousarray(outputs[:, -1, :])
    return outputs, h_last


# revision 20
# speedup vs baseline: 1.0107x; 1.0008x over previous
"""MinGRU Trainium2 kernel.

Reference computation (per batch b):
    z   = sigmoid(x @ Wz^T + bz)          [T, D]
    ht  = x @ Wh^T + bh                   [T, D]
    h_t = (1 - z_t) * h_{t-1} + z_t * ht_t    (scan over t, h_{-1} = h_prev)
    returns (outputs [B,T,D], h_last [B,D])

Strategy:
  - Data parallel: batch b -> core b (B = 8 = n_cores).
  - Everything on-device lives in TRANSPOSED layout [D, T] (features on
    partitions, time on the free axis):
      * matmuls compute Zpre^T = Wz @ x^T directly (lhsT = Wz^T block,
        rhs = x^T block), accumulating K=1024 in 8 steps of 128 in PSUM.
      * the recurrence is ONE tensor_tensor_scan instruction per
        [128, TC] tile: state = a*state + b along the free (time) axis.
  - The host pre-transposes x and the weights (numpy, free relative to
    device time) and transposes the [D, T] result back to [T, D].
  - Matmul inputs are cast to fp16 on the host (same PE speed as bf16,
    measured, but 10 mantissa bits; PSUM accumulation stays fp32);
    everything after the matmul is fp32.
"""

import time

import numpy as np
import ml_dtypes

import concourse.bass as bass  # noqa: F401  (bass types used via tile/bacc)
import concourse.mybir as mybir
import concourse.tile as tile
from contextlib import ExitStack
from concourse import bacc
from concourse.bass_utils import run_bass_kernel_spmd

# Problem sizes (hardcoded per contract).
B, T, D = 8, 4096, 1024
P = 128            # SBUF partitions
ND = D // P        # 8 output-feature blocks
NK = D // P        # 8 contraction blocks
TC = 512           # time-chunk = matmul free dim = one fp32 PSUM bank
NT = T // TC       # 8 time chunks

F32 = mybir.dt.float32
BF16 = mybir.dt.bfloat16

# Matmul input precision: "bf16", "fp16", or "f32r" (fp32 data, fast PE mode).
MM_DTYPE = "fp16"


def build(nt: int = NT, repeat: int = 1):
    """Build the Bass module. All 8 cores run the same program (SPMD).

    repeat > 1 re-runs the whole compute loop over the same data
    (timing-measurement variant; output is overwritten each pass)."""
    t_total = nt * TC
    nc = bacc.Bacc("TRN2", target_bir_lowering=False, debug=False, num_devices=8)

    MMDT = {
        "bf16": BF16,
        "fp16": mybir.dt.float16,
        "f32r": mybir.dt.float32r,
    }[MM_DTYPE]

    xT = nc.dram_tensor("xT", [D, t_total], MMDT, kind="ExternalInput")
    wzT = nc.dram_tensor("wzT", [D, D], MMDT, kind="ExternalInput")   # [k, d]
    whT = nc.dram_tensor("whT", [D, D], MMDT, kind="ExternalInput")   # [k, d]
    bz = nc.dram_tensor("bz", [P, ND], F32, kind="ExternalInput")     # bz[p,j] = Wz_b[j*128+p]
    nbz = nc.dram_tensor("nbz", [P, ND], F32, kind="ExternalInput")   # -Wz_b
    bh = nc.dram_tensor("bh", [P, ND], F32, kind="ExternalInput")
    h0 = nc.dram_tensor("h0", [P, ND], F32, kind="ExternalInput")
    outT = nc.dram_tensor("outT", [D, t_total], F32, kind="ExternalOutput")

    with tile.TileContext(nc) as tc, ExitStack() as ctx:
        consts = ctx.enter_context(tc.tile_pool(name="consts", bufs=1))
        wpool = ctx.enter_context(tc.tile_pool(name="w", bufs=1))
        xpool = ctx.enter_context(tc.tile_pool(name="x", bufs=3))
        zpool = ctx.enter_context(tc.tile_pool(name="z", bufs=4))
        apool = ctx.enter_context(tc.tile_pool(name="a", bufs=4))
        bpool = ctx.enter_context(tc.tile_pool(name="b", bufs=4))
        hpool = ctx.enter_context(tc.tile_pool(name="h", bufs=2 * ND))
        pspool = ctx.enter_context(tc.tile_pool(name="ps", bufs=8, space="PSUM"))

        # --- prefetch chunk 0 of x ahead of the weights (cuts startup) ---
        # one strided DMA per chunk: [p, kb, t] view of xT
        xTv = xT.rearrange("(kb p) t -> p kb t", p=P)
        xc0 = xpool.tile([P, NK, TC], MMDT, tag="xc")
        nc.sync.dma_start(xc0[:, :, :], xTv[:, :, 0:TC])

        # --- weights, transposed [k, d], one [128, D] tile per k-block ---
        wz_sb = []
        wh_sb = []
        for kb in range(NK):
            wz_t = wpool.tile([P, D], MMDT, tag=f"wz{kb}")
            nc.sync.dma_start(wz_t[:], wzT[kb * P:(kb + 1) * P, :])
            wz_sb.append(wz_t)
            wh_t = wpool.tile([P, D], MMDT, tag=f"wh{kb}")
            nc.sync.dma_start(wh_t[:], whT[kb * P:(kb + 1) * P, :])
            wh_sb.append(wh_t)

        # --- constants (small; after the bulk loads) ---
        bz_sb = consts.tile([P, ND], F32, tag="bz")
        nc.sync.dma_start(bz_sb[:], bz[:, :])
        nbz_sb = consts.tile([P, ND], F32, tag="nbz")
        nc.sync.dma_start(nbz_sb[:], nbz[:, :])
        bh_sb = consts.tile([P, ND], F32, tag="bh")
        nc.sync.dma_start(bh_sb[:], bh[:, :])
        h0_sb = consts.tile([P, ND], F32, tag="h0")
        nc.sync.dma_start(h0_sb[:], h0[:, :])

        # previous h tile per d-block (for scan chaining)
        h_prev_tiles = [None] * ND

        first = True
        for t in range(nt * repeat):
            t %= nt
            t0 = t * TC
            # x^T chunk: [128k, NK, TC]
            if first:
                xc = xc0
                first = False
            else:
                xc = xpool.tile([P, NK, TC], MMDT, tag="xc")
                nc.sync.dma_start(xc[:, :, :], xTv[:, :, t0:t0 + TC])

            for db in range(ND):
                dsl = slice(db * P, (db + 1) * P)
                # z-gate pre-activation: accumulate over k
                zp = pspool.tile([P, TC], F32, tag="ps")
                for kb in range(NK):
                    nc.tensor.matmul(
                        zp[:],
                        wz_sb[kb][:, dsl],
                        xc[:, kb, :],
                        start=(kb == 0),
                        stop=(kb == NK - 1),
                    )
                # h-tilde pre-activation
                hp = pspool.tile([P, TC], F32, tag="ps")
                for kb in range(NK):
                    nc.tensor.matmul(
                        hp[:],
                        wh_sb[kb][:, dsl],
                        xc[:, kb, :],
                        start=(kb == 0),
                        stop=(kb == NK - 1),
                    )

                # z = sigmoid(zp + bz);  a = 1 - z = sigmoid(-zp - bz)
                z_t = zpool.tile([P, TC], F32, tag="z")
                nc.scalar.activation(
                    z_t[:], zp[:], mybir.ActivationFunctionType.Sigmoid,
                    bias=bz_sb[:, db:db + 1], scale=1.0,
                )
                # a = 1 - z (exactly as the reference computes it)
                a_t = apool.tile([P, TC], F32, tag="a")
                nc.vector.tensor_scalar(
                    a_t[:], z_t[:], -1.0, 1.0,
                    op0=mybir.AluOpType.mult, op1=mybir.AluOpType.add,
                )
                # b = (hp + bh) * z
                b_t = bpool.tile([P, TC], F32, tag="b")
                nc.vector.scalar_tensor_tensor(
                    b_t[:], hp[:], bh_sb[:, db:db + 1], z_t[:],
                    op0=mybir.AluOpType.add, op1=mybir.AluOpType.mult,
                )
                # h_t = a*h_{t-1} + b  (scan along free/time axis)
                h_t = hpool.tile([P, TC], F32, tag="h")
                init = (
                    h0_sb[:, db:db + 1]
                    if h_prev_tiles[db] is None
                    else h_prev_tiles[db][:, TC - 1:TC]
                )
                nc.vector.tensor_tensor_scan(
                    h_t[:], a_t[:], b_t[:], init,
                    op0=mybir.AluOpType.mult, op1=mybir.AluOpType.add,
                )
                h_prev_tiles[db] = h_t
                nc.sync.dma_start(outT[dsl, t0:t0 + TC], h_t[:])

    nc.compile()
    return nc


def make_in_maps(x, h_prev, Wz_w, Wz_b, Wh_w, Wh_b):
    bf16 = {
        "bf16": ml_dtypes.bfloat16,
        "fp16": np.float16,
        "f32r": np.float32,
    }[MM_DTYPE]
    f32 = np.float32
    wzT = np.asarray(Wz_w).T.astype(bf16)          # [k, d], contiguous copy
    whT = np.asarray(Wh_w).T.astype(bf16)
    bz2 = np.asarray(Wz_b, f32).reshape(ND, P).T.copy()
    nbz2 = (-np.asarray(Wz_b, f32)).reshape(ND, P).T.copy()
    bh2 = np.asarray(Wh_b, f32).reshape(ND, P).T.copy()
    in_maps = []
    for b in range(B):
        in_maps.append({
            "xT": np.asarray(x[b]).T.astype(bf16),  # [D, T]
            "wzT": wzT,
            "whT": whT,
            "bz": bz2,
            "nbz": nbz2,
            "bh": bh2,
            "h0": np.asarray(h_prev[b], f32).reshape(ND, P).T.copy(),
        })
    return in_maps


def kernel(x, h_prev, Wz_w, Wz_b, Wh_w, Wh_b):
    x = np.asarray(x, np.float32)
    h_prev = np.asarray(h_prev, np.float32)
    Wz_w = np.asarray(Wz_w, np.float32)
    Wz_b = np.asarray(Wz_b, np.float32)
    Wh_w = np.asarray(Wh_w, np.float32)
    Wh_b = np.asarray(Wh_b, np.float32)
    in_maps = make_in_maps(x, h_prev, Wz_w, Wz_b, Wh_w, Wh_b)
    nc = build()
    res = None
    for attempt in range(2):
        try:
            res = run_bass_kernel_spmd(
                nc, in_maps, core_ids=list(range(B)), trace=False
            )
            break
        except Exception:
            if attempt == 1:
                raise
            # transient device errors have been observed to self-recover
            time.sleep(75)
    outputs = np.stack(
        [res.results[b]["outT"].T for b in range(B)]
    ).astype(np.float32)
    h_last = np.ascontiguousarray(outputs[:, -1, :])
    return outputs, h_last


# revision 22
# speedup vs baseline: 1.0151x; 1.0044x over previous
"""MinGRU Trainium2 kernel.

Reference computation (per batch b):
    z   = sigmoid(x @ Wz^T + bz)          [T, D]
    ht  = x @ Wh^T + bh                   [T, D]
    h_t = (1 - z_t) * h_{t-1} + z_t * ht_t    (scan over t, h_{-1} = h_prev)
    returns (outputs [B,T,D], h_last [B,D])

Strategy:
  - Data parallel: batch b -> core b (B = 8 = n_cores).
  - Everything on-device lives in TRANSPOSED layout [D, T] (features on
    partitions, time on the free axis):
      * matmuls compute Zpre^T = Wz @ x^T directly (lhsT = Wz^T block,
        rhs = x^T block), accumulating K=1024 in 8 steps of 128 in PSUM.
      * the recurrence is ONE tensor_tensor_scan instruction per
        [128, TC] tile: state = a*state + b along the free (time) axis.
  - The host pre-transposes x and the weights (numpy, free relative to
    device time) and transposes the [D, T] result back to [T, D].
  - Matmul inputs are cast to fp16 on the host (same PE speed as bf16,
    measured, but 10 mantissa bits; PSUM accumulation stays fp32);
    everything after the matmul is fp32.
"""

import time

import numpy as np
import ml_dtypes

import concourse.bass as bass  # noqa: F401  (bass types used via tile/bacc)
import concourse.mybir as mybir
import concourse.tile as tile
from contextlib import ExitStack
from concourse import bacc
from concourse.bass_utils import run_bass_kernel_spmd

# Problem sizes (hardcoded per contract).
B, T, D = 8, 4096, 1024
P = 128            # SBUF partitions
ND = D // P        # 8 output-feature blocks
NK = D // P        # 8 contraction blocks
TC = 512           # time-chunk = matmul free dim = one fp32 PSUM bank
NT = T // TC       # 8 time chunks

F32 = mybir.dt.float32
BF16 = mybir.dt.bfloat16

# Matmul input precision: "bf16", "fp16", or "f32r" (fp32 data, fast PE mode).
MM_DTYPE = "fp16"


def build(nt: int = NT, repeat: int = 1):
    """Build the Bass module. All 8 cores run the same program (SPMD).

    repeat > 1 re-runs the whole compute loop over the same data
    (timing-measurement variant; output is overwritten each pass)."""
    t_total = nt * TC
    nc = bacc.Bacc("TRN2", target_bir_lowering=False, debug=False, num_devices=8)

    MMDT = {
        "bf16": BF16,
        "fp16": mybir.dt.float16,
        "f32r": mybir.dt.float32r,
    }[MM_DTYPE]

    xT = nc.dram_tensor("xT", [D, t_total], MMDT, kind="ExternalInput")
    wzT = nc.dram_tensor("wzT", [D, D], MMDT, kind="ExternalInput")   # [k, d]
    whT = nc.dram_tensor("whT", [D, D], MMDT, kind="ExternalInput")   # [k, d]
    bz = nc.dram_tensor("bz", [P, ND], F32, kind="ExternalInput")     # bz[p,j] = Wz_b[j*128+p]
    bh = nc.dram_tensor("bh", [P, ND], F32, kind="ExternalInput")
    h0 = nc.dram_tensor("h0", [P, ND], F32, kind="ExternalInput")
    outT = nc.dram_tensor("outT", [D, t_total], F32, kind="ExternalOutput")

    with tile.TileContext(nc) as tc, ExitStack() as ctx:
        consts = ctx.enter_context(tc.tile_pool(name="consts", bufs=1))
        wpool = ctx.enter_context(tc.tile_pool(name="w", bufs=1))
        xpool = ctx.enter_context(tc.tile_pool(name="x", bufs=3))
        zpool = ctx.enter_context(tc.tile_pool(name="z", bufs=4))
        apool = ctx.enter_context(tc.tile_pool(name="a", bufs=4))
        bpool = ctx.enter_context(tc.tile_pool(name="b", bufs=4))
        hpool = ctx.enter_context(tc.tile_pool(name="h", bufs=2 * ND))
        pspool = ctx.enter_context(tc.tile_pool(name="ps", bufs=8, space="PSUM"))

        # --- PE warmup: throwaway matmuls on a zeroed tile so the HAM
        # clock-gate opens during the startup DMA wait ---
        wu = consts.tile([P, TC], MMDT, tag="wu")
        nc.vector.memset(wu[:], 0.0)
        wu_ps = pspool.tile([P, TC], F32, tag="ps")
        for i in range(16):
            nc.tensor.matmul(
                wu_ps[:], wu[:, 0:P], wu[:],
                start=(i == 0), stop=(i == 15),
            )

        # --- prefetch chunk 0 of x ahead of the weights (cuts startup) ---
        # one strided DMA per chunk: [p, kb, t] view of xT
        xTv = xT.rearrange("(kb p) t -> p kb t", p=P)
        xc0 = xpool.tile([P, NK, TC], MMDT, tag="xc")
        nc.sync.dma_start(xc0[:, :, :], xTv[:, :, 0:TC])

        # --- weights, transposed [k, d], one [128, D] tile per k-block ---
        wz_sb = []
        wh_sb = []
        for kb in range(NK):
            wz_t = wpool.tile([P, D], MMDT, tag=f"wz{kb}")
            nc.sync.dma_start(wz_t[:], wzT[kb * P:(kb + 1) * P, :])
            wz_sb.append(wz_t)
            wh_t = wpool.tile([P, D], MMDT, tag=f"wh{kb}")
            nc.sync.dma_start(wh_t[:], whT[kb * P:(kb + 1) * P, :])
            wh_sb.append(wh_t)

        # --- constants (small; after the bulk loads) ---
        bz_sb = consts.tile([P, ND], F32, tag="bz")
        nc.sync.dma_start(bz_sb[:], bz[:, :])
        bh_sb = consts.tile([P, ND], F32, tag="bh")
        nc.sync.dma_start(bh_sb[:], bh[:, :])
        h0_sb = consts.tile([P, ND], F32, tag="h0")
        nc.sync.dma_start(h0_sb[:], h0[:, :])

        # previous h tile per d-block (for scan chaining)
        h_prev_tiles = [None] * ND

        first = True
        for t in range(nt * repeat):
            t %= nt
            t0 = t * TC
            # x^T chunk: [128k, NK, TC]
            if first:
                xc = xc0
                first = False
            else:
                xc = xpool.tile([P, NK, TC], MMDT, tag="xc")
                nc.sync.dma_start(xc[:, :, :], xTv[:, :, t0:t0 + TC])

            for db in range(ND):
                dsl = slice(db * P, (db + 1) * P)
                # z-gate pre-activation: accumulate over k
                zp = pspool.tile([P, TC], F32, tag="ps")
                for kb in range(NK):
                    nc.tensor.matmul(
                        zp[:],
                        wz_sb[kb][:, dsl],
                        xc[:, kb, :],
                        start=(kb == 0),
                        stop=(kb == NK - 1),
                    )
                # h-tilde pre-activation
                hp = pspool.tile([P, TC], F32, tag="ps")
                for kb in range(NK):
                    nc.tensor.matmul(
                        hp[:],
                        wh_sb[kb][:, dsl],
                        xc[:, kb, :],
                        start=(kb == 0),
                        stop=(kb == NK - 1),
                    )

                # z = sigmoid(zp + bz);  a = 1 - z = sigmoid(-zp - bz)
                z_t = zpool.tile([P, TC], F32, tag="z")
                nc.scalar.activation(
                    z_t[:], zp[:], mybir.ActivationFunctionType.Sigmoid,
                    bias=bz_sb[:, db:db + 1], scale=1.0,
                )
                # a = 1 - z (exactly as the reference computes it)
                a_t = apool.tile([P, TC], F32, tag="a")
                nc.vector.tensor_scalar(
                    a_t[:], z_t[:], -1.0, 1.0,
                    op0=mybir.AluOpType.mult, op1=mybir.AluOpType.add,
                )
                # b = (hp + bh) * z
                b_t = bpool.tile([P, TC], F32, tag="b")
                nc.vector.scalar_tensor_tensor(
                    b_t[:], hp[:], bh_sb[:, db:db + 1], z_t[:],
                    op0=mybir.AluOpType.add, op1=mybir.AluOpType.mult,
                )
                # h_t = a*h_{t-1} + b  (scan along free/time axis)
                h_t = hpool.tile([P, TC], F32, tag="h")
                init = (
                    h0_sb[:, db:db + 1]
                    if h_prev_tiles[db] is None
                    else h_prev_tiles[db][:, TC - 1:TC]
                )
                nc.vector.tensor_tensor_scan(
                    h_t[:], a_t[:], b_t[:], init,
                    op0=mybir.AluOpType.mult, op1=mybir.AluOpType.add,
                )
                h_prev_tiles[db] = h_t
                nc.sync.dma_start(outT[dsl, t0:t0 + TC], h_t[:])

    nc.compile()
    return nc


def make_in_maps(x, h_prev, Wz_w, Wz_b, Wh_w, Wh_b):
    bf16 = {
        "bf16": ml_dtypes.bfloat16,
        "fp16": np.float16,
        "f32r": np.float32,
    }[MM_DTYPE]
    f32 = np.float32
    wzT = np.asarray(Wz_w).T.astype(bf16)          # [k, d], contiguous copy
    whT = np.asarray(Wh_w).T.astype(bf16)
    bz2 = np.asarray(Wz_b, f32).reshape(ND, P).T.copy()
    bh2 = np.asarray(Wh_b, f32).reshape(ND, P).T.copy()
    in_maps = []
    for b in range(B):
        in_maps.append({
            "xT": np.asarray(x[b]).T.astype(bf16),  # [D, T]
            "wzT": wzT,
            "whT": whT,
            "bz": bz2,
            "bh": bh2,
            "h0": np.asarray(h_prev[b], f32).reshape(ND, P).T.copy(),
        })
    return in_maps


def kernel(x, h_prev, Wz_w, Wz_b, Wh_w, Wh_b):
    x = np.asarray(x, np.float32)
    h_prev = np.asarray(h_prev, np.float32)
    Wz_w = np.asarray(Wz_w, np.float32)
    Wz_b = np.asarray(Wz_b, np.float32)
    Wh_w = np.asarray(Wh_w, np.float32)
    Wh_b = np.asarray(Wh_b, np.float32)
    in_maps = make_in_maps(x, h_prev, Wz_w, Wz_b, Wh_w, Wh_b)
    nc = build()
    res = None
    for attempt in range(2):
        try:
            res = run_bass_kernel_spmd(
                nc, in_maps, core_ids=list(range(B)), trace=False
            )
            break
        except Exception:
            if attempt == 1:
                raise
            # transient device errors have been observed to self-recover
            time.sleep(75)
    outputs = np.stack(
        [res.results[b]["outT"].T for b in range(B)]
    ).astype(np.float32)
    h_last = np.ascontiguousarray(outputs[:, -1, :])
    return outputs, h_last
